# revision 12
# baseline (speedup 1.0000x reference)
"""Trainium2 Bass kernel for a GRU encoder-decoder (KLCPD generator).

Model (see reference):
  past_emb = relu(past @ W_emb + b_emb)            [T,B,E]
  fut_emb  = relu(future @ W_emb + b_emb)          [T,B,E]
  _, h_T   = GRU_enc(past_emb, h0=0)
  hidden   = h_T + noise
  ys, _    = GRU_dec(shift(fut_emb), h0=hidden)
  out      = ys @ W_out + b_out                    [T,B,D]

Sharding: data-parallel over batch B=1024 across 8 NeuronCores
(B_local=128); all weights replicated; no collectives.

Per-core kernel layout decisions:
  * All matmul inputs are bf16 (fp32 accumulation in PSUM).
  * The GRU hidden state is kept *transposed* in SBUF as
    hT[p, k*128 + b] = h[b, k*128 + p]  (k = H-chunk 0..3), so the
    elementwise gate math produces, with zero extra transposes, exactly
    the stationary operand needed by the next step's matmuls.
  * Gate pre-activations are accumulated in four PSUM banks (r, z, hn,
    xn) in the same transposed layout; the input contribution
    x_emb @ W_ih of step t is accumulated into the same banks before the
    recurrent matmuls so it runs on the PE while step t-1's gate tail is
    still executing on ACT/DVE.
  * Embeddings for both inputs are precomputed once (PE transposes of
    the [128,128] input tiles + matmul + relu) into SBUF-resident
    transposed bf16 tiles embT[e][128, T*128].
"""

import os
from contextlib import ExitStack

import numpy as np

import concourse.bass as bass
import concourse.tile as tile
from concourse import bacc, bass_utils, masks, mybir

T, B, D, E, H = 64, 1024, 128, 256, 512
NCORES = 8
BL = B // NCORES  # 128
H3 = 3 * H
P = 128

f32 = mybir.dt.float32
bf16 = mybir.dt.bfloat16
AF = mybir.ActivationFunctionType
OP = mybir.AluOpType


def _mm(nc, out, lhsT, rhs, start, stop):
    nc.tensor.matmul(out, lhsT, rhs, start=start, stop=stop, skip_group_check=True)


def build_module(zero_bias: bool, t_steps: int = T, dump_h: bool = False):
    """Builds the per-core Bass module. Returns the compiled nc."""
    nc = bacc.Bacc("TRN2", target_bir_lowering=False, debug=False)
    dbg_h = None
    if dump_h:
        dbg_h = nc.dram_tensor("dbg_h", [2, t_steps, P, H], bf16, kind="ExternalOutput").ap()

    past = nc.dram_tensor("past", [t_steps, BL, D], f32, kind="ExternalInput").ap()
    fut = nc.dram_tensor("fut", [t_steps, BL, D], f32, kind="ExternalInput").ap()
    noise = nc.dram_tensor("noise", [BL, H], f32, kind="ExternalInput").ap()
    w_emb = nc.dram_tensor("w_emb", [D, E], f32, kind="ExternalInput").ap()
    b_emb = nc.dram_tensor("b_emb", [1, E], f32, kind="ExternalInput").ap()
    wd = {}
    for g in ("enc", "dec"):
        wd[g, "ih"] = nc.dram_tensor(f"w_ih_{g}", [E, H3], f32, kind="ExternalInput").ap()
        wd[g, "hh"] = nc.dram_tensor(f"w_hh_{g}", [H, H3], f32, kind="ExternalInput").ap()
        wd[g, "bih"] = nc.dram_tensor(f"b_ih_{g}", [1, H3], f32, kind="ExternalInput").ap()
        wd[g, "bhh"] = nc.dram_tensor(f"b_hh_{g}", [1, H3], f32, kind="ExternalInput").ap()
    w_out = nc.dram_tensor("w_out", [H, D], f32, kind="ExternalInput").ap()
    b_out = nc.dram_tensor("b_out", [1, D], f32, kind="ExternalInput").ap()
    out = nc.dram_tensor("out", [t_steps, BL, D], f32, kind="ExternalOutput").ap()

    with tile.TileContext(nc) as tc, ExitStack() as octx:
        wpool = octx.enter_context(tc.tile_pool(name="weights", bufs=1))

        # ---- constants -------------------------------------------------
        ident = wpool.tile([P, P], bf16)
        masks.make_identity(nc, ident[:])
        ones_row = wpool.tile([1, 512], bf16)
        nc.gpsimd.memset(ones_row[:], 1.0)

        # ---- weight preload + cast to bf16 -----------------------------
        whh = {}   # whh[g][k]: [128, H3]
        wih = {}   # wih[g][e]: [128, H3]
        biasx = {}  # [1, H3]  (b_ih + b_hh on r,z cols; b_ih on n cols)
        biashn = {}  # [1, 512] (b_hh n-part)
        with tc.tile_pool(name="wstage", bufs=2) as stage:
            wemb_bf = wpool.tile([P, E], bf16)
            st = stage.tile([P, E], f32, tag="s_emb")
            nc.sync.dma_start(st[:], w_emb[:, :])
            nc.vector.tensor_copy(wemb_bf[:], st[:])

            wout_bf = wpool.tile([P, 4 * P], bf16)  # col block k = W_out rows k
            st = stage.tile([P, 4 * P], f32, tag="s_out")
            for k in range(4):
                nc.sync.dma_start(st[:, k * P:(k + 1) * P], w_out[k * P:(k + 1) * P, :])
            nc.vector.tensor_copy(wout_bf[:], st[:])

            for g in ("enc", "dec"):
                wih[g] = []
                for e in range(2):
                    t_ = wpool.tile([P, H3], bf16, tag=f"wih_{g}_{e}")
                    st = stage.tile([P, H3], f32, tag="s_ih")
                    nc.sync.dma_start(st[:], wd[g, "ih"][e * P:(e + 1) * P, :])
                    nc.vector.tensor_copy(t_[:], st[:])
                    wih[g].append(t_)
                whh[g] = []
                for k in range(4):
                    t_ = wpool.tile([P, H3], bf16, tag=f"whh_{g}_{k}")
                    st = stage.tile([P, H3], f32, tag="s_hh")
                    nc.sync.dma_start(st[:], wd[g, "hh"][k * P:(k + 1) * P, :])
                    nc.scalar.copy(t_[:], st[:])
                    whh[g].append(t_)
                if not zero_bias:
                    sih = stage.tile([1, H3], f32, tag="s_bih")
                    shh = stage.tile([1, H3], f32, tag="s_bhh")
                    nc.sync.dma_start(sih[:], wd[g, "bih"][:, :])
                    nc.sync.dma_start(shh[:], wd[g, "bhh"][:, :])
                    bx = wpool.tile([1, H3], bf16, tag=f"biasx_{g}")
                    nc.vector.tensor_add(bx[:, 0:2 * H], sih[:, 0:2 * H], shh[:, 0:2 * H])
                    nc.vector.tensor_copy(bx[:, 2 * H:H3], sih[:, 2 * H:H3])
                    bh = wpool.tile([1, H], bf16, tag=f"biashn_{g}")
                    nc.vector.tensor_copy(bh[:], shh[:, 2 * H:H3])
                    biasx[g] = bx
                    biashn[g] = bh

            bemb_bf = None
            if not zero_bias:
                st = stage.tile([1, E], f32, tag="s_bemb")
                nc.sync.dma_start(st[:], b_emb[:, :])
                bemb_bf = wpool.tile([1, E], bf16)
                nc.vector.tensor_copy(bemb_bf[:], st[:])
                bout_bf = wpool.tile([1, D], bf16)
                st = stage.tile([1, D], f32, tag="s_bout")
                nc.sync.dma_start(st[:], b_out[:, :])
                nc.vector.tensor_copy(bout_bf[:], st[:])

            # noise, transposed: noiseT[p, k*128+b] = noise[b, k*128+p]
            noiseT = wpool.tile([P, H], bf16)
            st = stage.tile([P, H], f32, tag="s_noise")
            nc.sync.dma_start(st[:], noise[:, :])
            nbf = stage.tile([P, H], bf16, tag="s_noise_bf")
            nc.vector.tensor_copy(nbf[:], st[:])
            with tc.tile_pool(name="psum_noise", bufs=1, space="PSUM") as pn:
                pt = pn.tile([P, H], bf16)
                for k in range(4):
                    nc.tensor.transpose(pt[:, k * P:(k + 1) * P], nbf[:, k * P:(k + 1) * P], ident[:])
                nc.scalar.copy(noiseT[:], pt[:])

        # ---- embedding precompute --------------------------------------
        # embT[g][e][p, t*BL + b] = relu(x[t] @ W_emb + b_emb)[b, e*128+p]
        embT = {g: [wpool.tile([P, t_steps * BL], bf16, name=f"embT_{g}_{e}", tag=f"embT_{g}_{e}")
                    for e in range(2)]
                for g in ("enc", "dec")}
        n_grp = t_steps // 4
        with tc.tile_pool(name="estage", bufs=3) as ep, \
             tc.tile_pool(name="psum_emb", bufs=2, space="PSUM") as pep:
            for g, x_ap in (("enc", past), ("dec", fut)):
                for gi in range(n_grp):
                    xs = ep.tile([P, 4 * P], f32, tag="xs")
                    nc.sync.dma_start(
                        xs[:].rearrange("p (i d) -> p i d", i=4),
                        x_ap[4 * gi:4 * gi + 4].transpose([1, 0, 2]),
                    )
                    xbf = ep.tile([P, 4 * P], bf16, tag="xbf")
                    nc.vector.tensor_copy(xbf[:], xs[:])
                    ptr = pep.tile([P, 4 * P], bf16, tag="ptr")
                    for i in range(4):
                        nc.tensor.transpose(ptr[:, i * P:(i + 1) * P], xbf[:, i * P:(i + 1) * P], ident[:])
                    xT = ep.tile([P, 4 * P], bf16, tag="xT")
                    nc.scalar.copy(xT[:], ptr[:])
                    for e in range(2):
                        pe_ = pep.tile([P, 4 * P], f32, tag=f"pe{e}")
                        _mm(nc, pe_[:], wemb_bf[:, e * P:(e + 1) * P], xT[:],
                            start=True, stop=zero_bias)
                        if not zero_bias:
                            _mm(nc, pe_[:], bemb_bf[0:1, e * P:(e + 1) * P], ones_row[0:1, :],
                                start=False, stop=True)
                        dst = embT[g][e][:, gi * 4 * P:(gi + 1) * 4 * P]
                        if e == 0:
                            nc.scalar.activation(dst, pe_[:], AF.Relu)
                        else:
                            nc.vector.tensor_scalar_max(dst, pe_[:], 0.0)

        # ---- GRU loops --------------------------------------------------
        def gru_loop(g, is_dec, hT0, sb, pg, po_pool, potr_pool):
            """Runs t_steps of GRU g. hT0 = initial transposed state (or None).
            Returns final hT tile."""
            hT_prev = hT0
            pending_out = None  # (psum_o, t) for decoder output pipeline
            outf = None
            for t in range(t_steps):
                have_x = (not is_dec) or t > 0
                have_h = hT_prev is not None
                tcol = (t - 1) if is_dec else t

                have_xn = have_x or not zero_bias
                pr = pg.tile([P, H], f32, tag="pr")
                pz = pg.tile([P, H], f32, tag="pz")
                pxn = pg.tile([P, H], f32, name="pxn", tag="pxn") if have_xn else None
                phn = pg.tile([P, H], f32, name="phn", tag="phn") if have_h else None

                # One start=True per PSUM bank per step (the hardware's
                # pending-zero covers the whole 2KB bank); one stop on the
                # bank's last matmul. Track per-bank emitted/total counts.
                nbias = 0 if zero_bias else 1
                nxw = (2 if have_x else 0) + nbias
                nhw = 4 if have_h else 0
                totals = {id(pr): 4 * (nxw + nhw), id(pz): 4 * (nxw + nhw)}
                if pxn is not None:
                    totals[id(pxn)] = 4 * nxw
                if phn is not None:
                    totals[id(phn)] = 4 * (4 + nbias)
                emitted = {k: 0 for k in totals}

                def emit(bank, sl, lhsT, rhs):
                    emitted[id(bank)] += 1
                    _mm(nc, sl, lhsT, rhs,
                        start=emitted[id(bank)] == 1,
                        stop=emitted[id(bank)] == totals[id(bank)])

                # -- xW(t): input contributions (independent of h) --------
                if have_x:
                    lx = [embT[g][e][:, tcol * BL:(tcol + 1) * BL] for e in range(2)]
                else:
                    lx = None
                for bank, lo in ((pr, 0), (pz, H), (pxn, 2 * H)):
                    if bank is None:
                        continue
                    for m in range(4):
                        sl = bank[:, m * P:(m + 1) * P]
                        if not zero_bias:
                            emit(bank, sl, biasx[g][0:1, lo + m * P:lo + (m + 1) * P],
                                 ones_row[0:1, 0:P])
                        if have_x:
                            for e in range(2):
                                emit(bank, sl, wih[g][e][:, lo + m * P:lo + (m + 1) * P], lx[e])

                # -- decoder output pipeline for step t-1 -----------------
                if is_dec and pending_out is not None:
                    po_prev, tp = pending_out
                    osb = sb.tile([P, P], bf16, tag="osb")
                    nc.scalar.copy(osb[:], po_prev[:])
                    potr = potr_pool.tile([P, P], bf16, tag="otr")
                    nc.tensor.transpose(potr[:], osb[:], ident[:])
                    if tp % 4 == 0:
                        outf = sb.tile([P, 4 * P], f32, tag="outf")
                    nc.vector.tensor_copy(outf[:, (tp % 4) * P:(tp % 4 + 1) * P], potr[:])
                    if tp % 4 == 3:
                        nc.sync.dma_start(
                            out[tp - 3:tp + 1].transpose([1, 0, 2]),
                            outf[:].rearrange("p (i d) -> p i d", i=4),
                        )
                    pending_out = None

                # -- hW(t): recurrent matmuls; r bank first, hn, then z ---
                if have_h:
                    bank_order = ((pr, 0), (phn, 2 * H), (pz, H))
                    for bank, lo in bank_order:
                        for m in range(4):
                            sl = bank[:, m * P:(m + 1) * P]
                            if bank is phn and not zero_bias:
                                emit(bank, sl, biashn[g][0:1, m * P:(m + 1) * P],
                                     ones_row[0:1, 0:P])
                            for k in range(4):
                                emit(bank, sl, whh[g][k][:, lo + m * P:lo + (m + 1) * P],
                                     hT_prev[:, k * P:(k + 1) * P])

                # -- gate math (all in transposed layout) -----------------
                r_t = sb.tile([P, H], bf16, tag="r")
                nc.scalar.activation(r_t[:], pr[:], AF.Sigmoid)
                z_t = sb.tile([P, H], bf16, tag="z")
                nc.scalar.activation(z_t[:], pz[:], AF.Sigmoid)
                if have_h:
                    t1 = sb.tile([P, H], bf16, tag="t1")
                    nc.vector.tensor_mul(t1[:], r_t[:], phn[:])
                    if have_xn:
                        t2 = sb.tile([P, H], bf16, tag="t2")
                        nc.vector.tensor_add(t2[:], t1[:], pxn[:])
                    else:
                        t2 = t1
                    n_src = t2
                else:
                    n_src = pxn
                n_t = sb.tile([P, H], bf16, tag="n")
                nc.scalar.activation(n_t[:], n_src[:], AF.Tanh)
                p_t = sb.tile([P, H], bf16, tag="p")
                nc.vector.scalar_tensor_tensor(p_t[:], z_t[:], 1.0, n_t[:], OP.subtract, OP.mult)
                h_new = sb.tile([P, H], bf16, tag="h")
                if have_h:
                    w_t = sb.tile([P, H], bf16, tag="w")
                    nc.vector.tensor_mul(w_t[:], z_t[:], hT_prev[:])
                    nc.vector.tensor_sub(h_new[:], w_t[:], p_t[:])
                else:
                    nc.vector.tensor_scalar_mul(h_new[:], p_t[:], -1.0)
                hT_prev = h_new
                if dbg_h is not None:
                    nc.sync.dma_start(dbg_h[1 if is_dec else 0, t], h_new[:])

                # -- decoder per-step output matmul -----------------------
                if is_dec:
                    po = po_pool.tile([P, P], f32, tag="po")
                    if not zero_bias:
                        _mm(nc, po[:], bout_bf[0:1, :], ones_row[0:1, 0:P], start=True, stop=False)
                    for k in range(4):
                        _mm(nc, po[:], wout_bf[:, k * P:(k + 1) * P], h_new[:, k * P:(k + 1) * P],
                            start=zero_bias and k == 0, stop=k == 3)
                    pending_out = (po, t)

            # flush last decoder output
            if is_dec and pending_out is not None:
                po_prev, tp = pending_out
                osb = sb.tile([P, P], bf16, tag="osb")
                nc.scalar.copy(osb[:], po_prev[:])
                potr = potr_pool.tile([P, P], bf16, tag="otr")
                nc.tensor.transpose(potr[:], osb[:], ident[:])
                nc.vector.tensor_copy(outf[:, (tp % 4) * P:(tp % 4 + 1) * P], potr[:])
                nc.sync.dma_start(
                    out[tp - 3:tp + 1].transpose([1, 0, 2]),
                    outf[:].rearrange("p (i d) -> p i d", i=4),
                )
            return hT_prev

        with tc.tile_pool(name="gru_sb", bufs=3) as sb, \
             tc.tile_pool(name="psum_g", bufs=1, space="PSUM") as pg, \
             tc.tile_pool(name="psum_o", bufs=2, space="PSUM") as po_pool, \
             tc.tile_pool(name="psum_otr", bufs=2, space="PSUM") as potr_pool:
            hT_enc = gru_loop("enc", False, None, sb, pg, po_pool, potr_pool)
            hid = sb.tile([P, H], bf16, tag="h")
            nc.vector.tensor_add(hid[:], hT_enc[:], noiseT[:])
            gru_loop("dec", True, hid, sb, pg, po_pool, potr_pool)

    nc.compile()
    return nc


_CACHE = {}


def _get_module(zero_bias: bool):
    key = zero_bias
    if key not in _CACHE:
        _CACHE[key] = build_module(zero_bias)
    return _CACHE[key]


def kernel(past_input, future_input, noise,
           W_emb, b_emb,
           W_ih_enc, W_hh_enc, b_ih_enc, b_hh_enc,
           W_ih_dec, W_hh_dec, b_ih_dec, b_hh_dec,
           W_out, b_out):
    f = np.float32
    past_input = np.asarray(past_input, f)
    future_input = np.asarray(future_input, f)
    noise = np.asarray(noise, f)
    zero_bias = not any(
        np.any(np.asarray(b)) for b in (b_emb, b_ih_enc, b_hh_enc, b_ih_dec, b_hh_dec, b_out)
    )
    nc = _get_module(zero_bias)

    shared = {
        "w_emb": np.asarray(W_emb, f),
        "b_emb": np.asarray(b_emb, f).reshape(1, E),
        "w_ih_enc": np.asarray(W_ih_enc, f), "w_hh_enc": np.asarray(W_hh_enc, f),
        "b_ih_enc": np.asarray(b_ih_enc, f).reshape(1, H3),
        "b_hh_enc": np.asarray(b_hh_enc, f).reshape(1, H3),
        "w_ih_dec": np.asarray(W_ih_dec, f), "w_hh_dec": np.asarray(W_hh_dec, f),
        "b_ih_dec": np.asarray(b_ih_dec, f).reshape(1, H3),
        "b_hh_dec": np.asarray(b_hh_dec, f).reshape(1, H3),
        "w_out": np.asarray(W_out, f),
        "b_out": np.asarray(b_out, f).reshape(1, D),
    }
    in_maps = []
    for c in range(NCORES):
        sl = slice(c * BL, (c + 1) * BL)
        m = dict(shared)
        m["past"] = np.ascontiguousarray(past_input[:, sl, :])
        m["fut"] = np.ascontiguousarray(future_input[:, sl, :])
        m["noise"] = np.ascontiguousarray(noise[sl, :])
        in_maps.append(m)

    res = bass_utils.run_bass_kernel_spmd(nc, in_maps, core_ids=list(range(NCORES)))
    return np.concatenate([r["out"] for r in res.results], axis=1)
